# revision 7
# baseline (speedup 1.0000x reference)
"""3x3 same-conv (NHWC, 32x56x56x128 -> 32x56x56x256) + bias + ReLU on 8 TRN2 cores.

Strategy: data-parallel over batch (4 images/core). Per core, the conv is
9 shifted matmuls accumulated in PSUM with Cin=128 as the contraction dim:
  out[q, cout] = relu( sum_tap XpT[:, q+off_tap]^T @ W[tap] + b )
where XpT is the zero-padded image held transposed in SBUF ([cin, 58*58]
flat padded pixels), built once per image via PE transposes. Anchors q are
processed 128 at a time over the flat padded index space; the 2/58 pad
columns produce garbage rows that are simply never stored. Matmuls run as
float32r (full-rate on TRN2 for N>=256) on bitcast fp32 data.
"""

import os
from contextlib import ExitStack

import numpy as np

import concourse.bass as bass
import concourse.bacc as bacc
import concourse.mybir as mybir
import concourse.tile as tile
from concourse.bass_utils import run_bass_kernel_spmd
from concourse.masks import make_identity

N_CORES = 8
B, H, W, CIN, COUT = 32, 56, 56, 128, 256
BPC = B // N_CORES            # images per core
S = W + 2                     # padded width (58)
PIMG = S * S                  # padded pixels per image (3364)
ANCH0 = S + 1                 # first valid anchor (59)
SPAN = (H * S + W) - ANCH0 + 1  # 3246 anchor positions covering all valid pixels
GROUPS = (SPAN + 127) // 128  # 26 anchor groups per image
SLAB_W = PIMG + 128           # per-image slab width incl. zero slop
RPC = 2                       # image rows per transpose chunk
CHUNK_PIX = RPC * W           # 112
NCHUNK = H // RPC             # 28

TAP_OFFS = [(dh - 1) * S + (dw - 1) for dh in range(3) for dw in range(3)]
F32 = mybir.dt.float32
F32R = mybir.dt.float32r
F16 = mybir.dt.float16

LAST_RESULTS = None


def _build(with_bias: bool):
    nc = bacc.Bacc("TRN2", target_bir_lowering=False, debug=False)
    x_h = nc.declare_dram_parameter("prev_a", [BPC, H, W, CIN], F32, isOutput=False)
    w_h = nc.declare_dram_parameter("filter_w", [3, 3, CIN, COUT], F32, isOutput=False)
    b_h = nc.declare_dram_parameter("filter_b", [1, 1, 1, COUT], F32, isOutput=False)
    y_h = nc.declare_dram_parameter("out", [BPC, H, W, COUT], F32, isOutput=True)
    x_ap, w_ap, b_ap, y_ap = x_h.ap(), w_h.ap(), b_h.ap(), y_h.ap()

    with tile.TileContext(nc) as tc, ExitStack() as ctx:
        const_pool = ctx.enter_context(tc.tile_pool(name="const", bufs=1))
        xslab_pool = ctx.enter_context(tc.tile_pool(name="xslab", bufs=1))
        stage_pool = ctx.enter_context(tc.tile_pool(name="stage", bufs=2))
        out_pool = ctx.enter_context(tc.tile_pool(name="outsb", bufs=6))
        psum_mm = ctx.enter_context(
            tc.tile_pool(name="psmm", bufs=4, space=bass.MemorySpace.PSUM)
        )
        psum_tp = ctx.enter_context(
            tc.tile_pool(name="pstp", bufs=4, space=bass.MemorySpace.PSUM)
        )

        # Weights: [3,3,128,256] -> SBUF [cin=128, tap*256], rounded to fp32r
        wstage = const_pool.tile([CIN, 9 * COUT], F32, tag="wstage")
        nc.sync.dma_start(
            out=wstage[:].rearrange("k (t n) -> k t n", t=9),
            in_=w_ap.rearrange("a b k n -> (a b) k n").transpose([1, 0, 2]),
        )
        wslab = const_pool.tile([CIN, 9 * COUT], F16, tag="wslab")
        nc.vector.tensor_copy(wslab[:], wstage[:])

        identity = const_pool.tile([CHUNK_PIX, CHUNK_PIX], F16, tag="ident")
        make_identity(nc, identity[:])

        if with_bias:
            bias_st = const_pool.tile([1, COUT], F32, tag="bias_st")
            nc.sync.dma_start(
                out=bias_st[:], in_=b_ap.rearrange("a b c n -> (a b c) n")
            )
            bias_sb = const_pool.tile([1, COUT], F16, tag="bias")
            nc.vector.tensor_copy(bias_sb[:], bias_st[:])
            ones_sb = const_pool.tile([1, 128], F16, tag="ones")
            nc.gpsimd.memset(ones_sb[:], 1.0)

        # Per-image transposed padded slabs [cin, 58*58 (+slop)]
        xslabs = []
        for i in range(BPC):
            sl = xslab_pool.tile([CIN, SLAB_W], F16, tag=f"xs{i}")
            xslabs.append(sl)
            nc.gpsimd.memset(sl[:, 0:S], 0.0)  # top pad row
            nc.gpsimd.memset(sl[:, (H + 1) * S : PIMG], 0.0)  # bottom pad row
            mid = sl[:, S : (H + 1) * S].rearrange("p (r c) -> p r c", c=S)
            nc.gpsimd.memset(mid[:, :, 0:1], 0.0)  # left pad col
            nc.gpsimd.memset(mid[:, :, S - 1 : S], 0.0)  # right pad col
            nc.gpsimd.memset(sl[:, PIMG:SLAB_W], 0.0)  # slop

        # Slab build steps (load image, PE-transpose 2-row chunks, copy into
        # slab), emitted lazily so they interleave with prior image's matmuls
        def emit_load(i):
            stg = stage_pool.tile([CHUNK_PIX, NCHUNK * CIN], F16, tag="stage")
            src = (
                x_ap[i]
                .rearrange("h w c -> (h w) c")
                .rearrange("(n p) c -> n p c", p=CHUNK_PIX)
                .transpose([1, 0, 2])
            )
            nc.gpsimd.dma_start(
                out=stg[:].rearrange("p (n c) -> p n c", n=NCHUNK), in_=src
            )
            return stg

        def emit_transpose(i, stg, cidx):
            pst = psum_tp.tile([CIN, CHUNK_PIX], F16, tag="pst")
            nc.tensor.transpose(
                pst[:], stg[:, cidx * CIN : (cidx + 1) * CIN], identity[:]
            )
            dst = (
                xslabs[i][:, (RPC * cidx + 1) * S : (RPC * cidx + 1 + RPC) * S]
                .rearrange("p (r c) -> p r c", c=S)[:, :, 1 : 1 + W]
            )
            nc.scalar.activation(
                dst,
                pst[:].rearrange("p (r c) -> p r c", c=W),
                mybir.ActivationFunctionType.Copy,
            )

        def emit_group(i, g):
            q0 = ANCH0 + g * 128
            ps = psum_mm.tile([128, COUT], F32, tag="psmm")
            for t in range(9):
                w0 = q0 + TAP_OFFS[t]
                nc.tensor.matmul(
                    ps[:],
                    xslabs[i][:, w0 : w0 + 128],
                    wslab[:, t * COUT : (t + 1) * COUT],
                    start=(t == 0),
                    stop=(t == 8 and not with_bias),
                )
            if with_bias:
                nc.tensor.matmul(
                    ps[:], ones_sb[:1, :], bias_sb[:1, :], start=False, stop=True
                )
            osb = out_pool.tile([128, COUT], F32, tag="osb")
            nc.vector.tensor_scalar_max(osb[:], ps[:], 0.0)
            # Store valid anchor runs (rows of the unpadded output)
            for hp in range(q0 // S, (q0 + 127) // S + 1):
                if hp < 1 or hp > H:
                    continue
                lo = max(q0, hp * S + 1)
                hi = min(q0 + 127, hp * S + W)
                if lo > hi:
                    continue
                plo = lo - q0
                wlo = lo - hp * S - 1
                cnt = hi - lo + 1
                nc.sync.dma_start(
                    out=y_ap[i, hp - 1, wlo : wlo + cnt, :],
                    in_=osb[plo : plo + cnt, :],
                )

        # Image 0's slab is built up front; image i+1's transposes are
        # interleaved between image i's matmul groups so the PE never waits
        # on a bulk transpose phase.
        stg0 = emit_load(0)
        nxt = emit_load(1)
        for c in range(NCHUNK):
            emit_transpose(0, stg0, c)
        for i in range(BPC):
            done = 0
            for g in range(GROUPS):
                emit_group(i, g)
                if i + 1 < BPC:
                    # spread the 28 transposes of image i+1 over the groups
                    want = (g + 1) * NCHUNK // GROUPS
                    while done < want:
                        emit_transpose(i + 1, nxt, done)
                        done += 1
            if i + 1 < BPC:
                while done < NCHUNK:
                    emit_transpose(i + 1, nxt, done)
                    done += 1
                if i + 2 < BPC:
                    nxt = emit_load(i + 2)

    nc.compile()
    return nc


_CACHE = {}


def _get_nc(with_bias: bool):
    if with_bias not in _CACHE:
        _CACHE[with_bias] = _build(with_bias)
    return _CACHE[with_bias]


def kernel(prev_a, filter_w, filter_b):
    global LAST_RESULTS
    prev_a = np.ascontiguousarray(prev_a, dtype=np.float32)
    filter_w = np.ascontiguousarray(filter_w, dtype=np.float32)
    filter_b = np.ascontiguousarray(filter_b, dtype=np.float32).reshape(1, 1, 1, COUT)
    with_bias = bool(np.any(filter_b))
    nc = _get_nc(with_bias)
    in_maps = [
        {
            "prev_a": prev_a[c * BPC : (c + 1) * BPC],
            "filter_w": filter_w,
            "filter_b": filter_b,
        }
        for c in range(N_CORES)
    ]
    trace = os.environ.get("KERNEL_TRACE") == "1"
    res = run_bass_kernel_spmd(nc, in_maps, list(range(N_CORES)), trace=trace)
    LAST_RESULTS = res
    return np.concatenate([res.results[c]["out"] for c in range(N_CORES)], axis=0)


# revision 9
# speedup vs baseline: 1.1740x; 1.1740x over previous
"""3x3 same-conv (NHWC, 32x56x56x128 -> 32x56x56x256) + bias + ReLU on 8 TRN2 cores.

Strategy: data-parallel over batch (4 images/core). Per core, the conv is
9 shifted matmuls accumulated in PSUM with Cin=128 as the contraction dim:
  out[q, cout] = relu( sum_tap XpT[:, q+off_tap]^T @ W[tap] + b )
where XpT is the zero-padded image held transposed in SBUF ([cin, 58*58]
flat padded pixels), built once per image via PE transposes. Anchors q are
processed 128 at a time over the flat padded index space; the 2/58 pad
columns produce garbage rows that are simply never stored. Matmuls run as
float32r (full-rate on TRN2 for N>=256) on bitcast fp32 data.
"""

import os
from contextlib import ExitStack

import numpy as np

import concourse.bass as bass
import concourse.bacc as bacc
import concourse.mybir as mybir
import concourse.tile as tile
from concourse.bass_utils import run_bass_kernel_spmd
from concourse.masks import make_identity

N_CORES = 8
B, H, W, CIN, COUT = 32, 56, 56, 128, 256
BPC = B // N_CORES            # images per core
S = W + 2                     # padded width (58)
PIMG = S * S                  # padded pixels per image (3364)
ANCH0 = S + 1                 # first valid anchor (59)
SPAN = (H * S + W) - ANCH0 + 1  # 3246 anchor positions covering all valid pixels
GROUPS = (SPAN + 127) // 128  # 26 anchor groups per image
SLAB_W = PIMG + 128           # per-image slab width incl. zero slop
RPC = 2                       # image rows per transpose chunk
CHUNK_PIX = RPC * W           # 112
NCHUNK = H // RPC             # 28

TAP_OFFS = [(dh - 1) * S + (dw - 1) for dh in range(3) for dw in range(3)]
F32 = mybir.dt.float32
F32R = mybir.dt.float32r
F16 = mybir.dt.float16

LAST_RESULTS = None

if os.environ.get("KERNEL_LDWOPT") == "1":
    # Experiment: let walrus run its LDWEIGHTS optimization pass.
    import concourse.bass_utils as _bu

    _orig_run_command = _bu.run_command

    def _patched_run_command(argv, **kw):
        argv = [
            "--enable-ldw-opt=true" if a == "--enable-ldw-opt=false" else a
            for a in argv
        ]
        return _orig_run_command(argv, **kw)

    _bu.run_command = _patched_run_command


def _build(with_bias: bool):
    nc = bacc.Bacc("TRN2", target_bir_lowering=False, debug=False)
    x_h = nc.declare_dram_parameter("prev_a", [BPC, H, W, CIN], F32, isOutput=False)
    w_h = nc.declare_dram_parameter("filter_w", [3, 3, CIN, COUT], F32, isOutput=False)
    b_h = nc.declare_dram_parameter("filter_b", [1, 1, 1, COUT], F32, isOutput=False)
    y_h = nc.declare_dram_parameter("out", [BPC, H, W, COUT], F32, isOutput=True)
    x_ap, w_ap, b_ap, y_ap = x_h.ap(), w_h.ap(), b_h.ap(), y_h.ap()

    with tile.TileContext(nc) as tc, ExitStack() as ctx:
        const_pool = ctx.enter_context(tc.tile_pool(name="const", bufs=1))
        xslab_pool = ctx.enter_context(tc.tile_pool(name="xslab", bufs=1))
        stage_pool = ctx.enter_context(tc.tile_pool(name="stage", bufs=2))
        out_pool = ctx.enter_context(tc.tile_pool(name="outsb", bufs=12))
        psum_mm = ctx.enter_context(
            tc.tile_pool(name="psmm", bufs=4, space=bass.MemorySpace.PSUM)
        )
        psum_tp = ctx.enter_context(
            tc.tile_pool(name="pstp", bufs=4, space=bass.MemorySpace.PSUM)
        )

        # Weights: [3,3,128,256] -> SBUF [cin=128, tap*256], rounded to fp32r
        wstage = const_pool.tile([CIN, 9 * COUT], F32, tag="wstage")
        nc.sync.dma_start(
            out=wstage[:].rearrange("k (t n) -> k t n", t=9),
            in_=w_ap.rearrange("a b k n -> (a b) k n").transpose([1, 0, 2]),
        )
        wslab = const_pool.tile([CIN, 9 * COUT], F16, tag="wslab")
        nc.vector.tensor_copy(wslab[:], wstage[:])

        identity = const_pool.tile([CHUNK_PIX, CHUNK_PIX], F16, tag="ident")
        make_identity(nc, identity[:])

        if with_bias:
            bias_st = const_pool.tile([1, COUT], F32, tag="bias_st")
            nc.sync.dma_start(
                out=bias_st[:], in_=b_ap.rearrange("a b c n -> (a b c) n")
            )
            bias_sb = const_pool.tile([1, COUT], F16, tag="bias")
            nc.vector.tensor_copy(bias_sb[:], bias_st[:])
            ones_sb = const_pool.tile([1, 128], F16, tag="ones")
            nc.gpsimd.memset(ones_sb[:], 1.0)

        # Per-image transposed padded slabs [cin, 58*58 (+slop)]
        xslabs = []
        for i in range(BPC):
            sl = xslab_pool.tile([CIN, SLAB_W], F16, tag=f"xs{i}")
            xslabs.append(sl)
            nc.gpsimd.memset(sl[:, 0:S], 0.0)  # top pad row
            nc.gpsimd.memset(sl[:, (H + 1) * S : PIMG], 0.0)  # bottom pad row
            mid = sl[:, S : (H + 1) * S].rearrange("p (r c) -> p r c", c=S)
            nc.gpsimd.memset(mid[:, :, 0:1], 0.0)  # left pad col
            nc.gpsimd.memset(mid[:, :, S - 1 : S], 0.0)  # right pad col
            nc.gpsimd.memset(sl[:, PIMG:SLAB_W], 0.0)  # slop

        # Slab build steps (load image, PE-transpose 2-row chunks, copy into
        # slab), emitted lazily so they interleave with prior image's matmuls
        def emit_load(i):
            stg = stage_pool.tile([CHUNK_PIX, NCHUNK * CIN], F16, tag="stage")
            src = (
                x_ap[i]
                .rearrange("h w c -> (h w) c")
                .rearrange("(n p) c -> n p c", p=CHUNK_PIX)
                .transpose([1, 0, 2])
            )
            nc.gpsimd.dma_start(
                out=stg[:].rearrange("p (n c) -> p n c", n=NCHUNK), in_=src
            )
            return stg

        def emit_transpose(i, stg, cidx):
            pst = psum_tp.tile([CIN, CHUNK_PIX], F16, tag="pst")
            nc.tensor.transpose(
                pst[:], stg[:, cidx * CIN : (cidx + 1) * CIN], identity[:]
            )
            dst = (
                xslabs[i][:, (RPC * cidx + 1) * S : (RPC * cidx + 1 + RPC) * S]
                .rearrange("p (r c) -> p r c", c=S)[:, :, 1 : 1 + W]
            )
            nc.scalar.activation(
                dst,
                pst[:].rearrange("p (r c) -> p r c", c=W),
                mybir.ActivationFunctionType.Copy,
            )

        def emit_group(i, g):
            q0 = ANCH0 + g * 128
            ps = psum_mm.tile([128, COUT], F32, tag="psmm")
            for t in range(9):
                w0 = q0 + TAP_OFFS[t]
                nc.tensor.matmul(
                    ps[:],
                    xslabs[i][:, w0 : w0 + 128],
                    wslab[:, t * COUT : (t + 1) * COUT],
                    start=(t == 0),
                    stop=(t == 8 and not with_bias),
                )
            if with_bias:
                nc.tensor.matmul(
                    ps[:], ones_sb[:1, :], bias_sb[:1, :], start=False, stop=True
                )
            osb = out_pool.tile([128, COUT], F32, tag="osb")
            nc.vector.tensor_scalar_max(osb[:], ps[:], 0.0)
            # Store valid anchor runs (rows of the unpadded output)
            for hp in range(q0 // S, (q0 + 127) // S + 1):
                if hp < 1 or hp > H:
                    continue
                lo = max(q0, hp * S + 1)
                hi = min(q0 + 127, hp * S + W)
                if lo > hi:
                    continue
                plo = lo - q0
                wlo = lo - hp * S - 1
                cnt = hi - lo + 1
                nc.sync.dma_start(
                    out=y_ap[i, hp - 1, wlo : wlo + cnt, :],
                    in_=osb[plo : plo + cnt, :],
                )

        # Image 0's slab is built up front; image i+1's transposes are
        # interleaved between image i's matmul groups so the PE never waits
        # on a bulk transpose phase.
        stg0 = emit_load(0)
        nxt = emit_load(1)
        for c in range(NCHUNK):
            emit_transpose(0, stg0, c)
        for i in range(BPC):
            done = 0
            for g in range(GROUPS):
                emit_group(i, g)
                if i + 1 < BPC:
                    # spread the 28 transposes of image i+1 over the groups
                    want = (g + 1) * NCHUNK // GROUPS
                    while done < want:
                        emit_transpose(i + 1, nxt, done)
                        done += 1
            if i + 1 < BPC:
                while done < NCHUNK:
                    emit_transpose(i + 1, nxt, done)
                    done += 1
                if i + 2 < BPC:
                    nxt = emit_load(i + 2)

    nc.compile()
    return nc


_CACHE = {}


def _get_nc(with_bias: bool):
    if with_bias not in _CACHE:
        _CACHE[with_bias] = _build(with_bias)
    return _CACHE[with_bias]


def kernel(prev_a, filter_w, filter_b):
    global LAST_RESULTS
    prev_a = np.ascontiguousarray(prev_a, dtype=np.float32)
    filter_w = np.ascontiguousarray(filter_w, dtype=np.float32)
    filter_b = np.ascontiguousarray(filter_b, dtype=np.float32).reshape(1, 1, 1, COUT)
    with_bias = bool(np.any(filter_b))
    nc = _get_nc(with_bias)
    in_maps = [
        {
            "prev_a": prev_a[c * BPC : (c + 1) * BPC],
            "filter_w": filter_w,
            "filter_b": filter_b,
        }
        for c in range(N_CORES)
    ]
    trace = os.environ.get("KERNEL_TRACE") == "1"
    res = run_bass_kernel_spmd(nc, in_maps, list(range(N_CORES)), trace=trace)
    LAST_RESULTS = res
    return np.concatenate([res.results[c]["out"] for c in range(N_CORES)], axis=0)


# revision 12
# speedup vs baseline: 2.1285x; 1.8131x over previous
"""3x3 same-conv (NHWC, 32x56x56x128 -> 32x56x56x256) + bias + ReLU on 8 TRN2 cores.

Strategy: data-parallel over batch (4 images/core). Per core, the conv is
9 shifted matmuls accumulated in PSUM with Cin=128 as the contraction dim:
  out[q, cout] = relu( sum_tap XpT[:, q+off_tap]^T @ W[tap] + b )
where XpT is the zero-padded image held transposed in SBUF ([cin, 58*58]
flat padded pixels), built once per image via PE transposes. Anchors q are
processed 128 at a time over the flat padded index space; the 2/58 pad
columns produce garbage rows that are simply never stored. Matmuls run as
float32r (full-rate on TRN2 for N>=256) on bitcast fp32 data.
"""

import os
from contextlib import ExitStack

import numpy as np

import concourse.bass as bass
import concourse.bacc as bacc
import concourse.mybir as mybir
import concourse.tile as tile
from concourse.bass_utils import run_bass_kernel_spmd
from concourse.masks import make_identity

N_CORES = 8
B, H, W, CIN, COUT = 32, 56, 56, 128, 256
BPC = B // N_CORES            # images per core
S = W + 2                     # padded width (58)
PIMG = S * S                  # padded pixels per image (3364)
ANCH0 = S + 1                 # first valid anchor (59)
GROUPS = H // 2               # 28 row-pair anchor groups per image
GM = 2 * S                    # anchors per group (116: 2 padded rows, 4 junk)
SLAB_W = PIMG + 128           # per-image slab width incl. zero slop
RPC = 2                       # image rows per transpose chunk
CHUNK_PIX = RPC * W           # 112
NCHUNK = H // RPC             # 28

TAP_OFFS = [(dh - 1) * S + (dw - 1) for dh in range(3) for dw in range(3)]
F32 = mybir.dt.float32
F32R = mybir.dt.float32r
F16 = mybir.dt.float16

LAST_RESULTS = None

if os.environ.get("KERNEL_LDWOPT") == "1":
    # Experiment: let walrus run its LDWEIGHTS optimization pass.
    import concourse.bass_utils as _bu

    _orig_run_command = _bu.run_command

    def _patched_run_command(argv, **kw):
        argv = [
            "--enable-ldw-opt=true" if a == "--enable-ldw-opt=false" else a
            for a in argv
        ]
        return _orig_run_command(argv, **kw)

    _bu.run_command = _patched_run_command


def _build(with_bias: bool):
    nc = bacc.Bacc("TRN2", target_bir_lowering=False, debug=False)
    x_h = nc.declare_dram_parameter("prev_a", [BPC, H, W, CIN], F32, isOutput=False)
    w_h = nc.declare_dram_parameter("filter_w", [3, 3, CIN, COUT], F32, isOutput=False)
    b_h = nc.declare_dram_parameter("filter_b", [1, 1, 1, COUT], F32, isOutput=False)
    y_h = nc.declare_dram_parameter("out", [BPC, H, W, COUT], F32, isOutput=True)
    x_ap, w_ap, b_ap, y_ap = x_h.ap(), w_h.ap(), b_h.ap(), y_h.ap()

    with tile.TileContext(nc) as tc, ExitStack() as ctx:
        const_pool = ctx.enter_context(tc.tile_pool(name="const", bufs=1))
        xslab_pool = ctx.enter_context(tc.tile_pool(name="xslab", bufs=1))
        stage_pool = ctx.enter_context(tc.tile_pool(name="stage", bufs=2))
        out_pool = ctx.enter_context(tc.tile_pool(name="outsb", bufs=2))
        psum_mm = ctx.enter_context(
            tc.tile_pool(name="psmm", bufs=4, space=bass.MemorySpace.PSUM)
        )
        psum_tp = ctx.enter_context(
            tc.tile_pool(name="pstp", bufs=4, space=bass.MemorySpace.PSUM)
        )

        # Weights: [3,3,128,256] -> SBUF [cin=128, tap*256], rounded to fp32r
        wstage = const_pool.tile([CIN, 9 * COUT], F32, tag="wstage")
        nc.sync.dma_start(
            out=wstage[:].rearrange("k (t n) -> k t n", t=9),
            in_=w_ap.rearrange("a b k n -> (a b) k n").transpose([1, 0, 2]),
        )
        wslab = const_pool.tile([CIN, 9 * COUT], F16, tag="wslab")
        nc.vector.tensor_copy(wslab[:], wstage[:])

        identity = const_pool.tile([CHUNK_PIX, CHUNK_PIX], F16, tag="ident")
        make_identity(nc, identity[:])

        if with_bias:
            bias_st = const_pool.tile([1, COUT], F32, tag="bias_st")
            nc.sync.dma_start(
                out=bias_st[:], in_=b_ap.rearrange("a b c n -> (a b c) n")
            )
            bias_sb = const_pool.tile([1, COUT], F16, tag="bias")
            nc.vector.tensor_copy(bias_sb[:], bias_st[:])
            ones_sb = const_pool.tile([1, 128], F16, tag="ones")
            nc.gpsimd.memset(ones_sb[:], 1.0)

        # Per-image transposed padded slabs [cin, 58*58 (+slop)]
        xslabs = []
        for i in range(BPC):
            sl = xslab_pool.tile([CIN, SLAB_W], F16, tag=f"xs{i}")
            xslabs.append(sl)
            nc.gpsimd.memset(sl[:, 0:S], 0.0)  # top pad row
            nc.gpsimd.memset(sl[:, (H + 1) * S : PIMG], 0.0)  # bottom pad row
            mid = sl[:, S : (H + 1) * S].rearrange("p (r c) -> p r c", c=S)
            nc.gpsimd.memset(mid[:, :, 0:1], 0.0)  # left pad col
            nc.gpsimd.memset(mid[:, :, S - 1 : S], 0.0)  # right pad col
            nc.gpsimd.memset(sl[:, PIMG:SLAB_W], 0.0)  # slop

        # Slab build steps (load image, PE-transpose 2-row chunks, copy into
        # slab), emitted lazily so they interleave with prior image's matmuls
        def emit_load(i):
            stg = stage_pool.tile([CHUNK_PIX, NCHUNK * CIN], F16, tag="stage")
            src = (
                x_ap[i]
                .rearrange("h w c -> (h w) c")
                .rearrange("(n p) c -> n p c", p=CHUNK_PIX)
                .transpose([1, 0, 2])
            )
            nc.gpsimd.dma_start(
                out=stg[:].rearrange("p (n c) -> p n c", n=NCHUNK), in_=src
            )
            return stg

        def emit_transpose(i, stg, cidx):
            pst = psum_tp.tile([CIN, CHUNK_PIX], F16, tag="pst")
            nc.tensor.transpose(
                pst[:], stg[:, cidx * CIN : (cidx + 1) * CIN], identity[:]
            )
            dst = (
                xslabs[i][:, (RPC * cidx + 1) * S : (RPC * cidx + 1 + RPC) * S]
                .rearrange("p (r c) -> p r c", c=S)[:, :, 1 : 1 + W]
            )
            nc.scalar.activation(
                dst,
                pst[:].rearrange("p (r c) -> p r c", c=W),
                mybir.ActivationFunctionType.Copy,
            )

        def emit_group(i, g, oslab):
            # anchors = 116 contiguous padded positions covering output rows
            # (2g, 2g+1); partitions 56,57,114,115 are pad junk (never stored)
            q0 = (2 * g + 1) * S + 1
            ps = psum_mm.tile([GM, COUT], F32, tag="psmm")
            for t in range(9):
                w0 = q0 + TAP_OFFS[t]
                nc.tensor.matmul(
                    ps[:],
                    xslabs[i][:, w0 : w0 + GM],
                    wslab[:, t * COUT : (t + 1) * COUT],
                    start=(t == 0),
                    stop=(t == 8 and not with_bias),
                )
            if with_bias:
                nc.tensor.matmul(
                    ps[:], ones_sb[:1, :GM], bias_sb[:1, :], start=False, stop=True
                )
            nc.vector.tensor_scalar_max(
                oslab[:, g * COUT : (g + 1) * COUT], ps[:], 0.0
            )

        # Image 0's slab is built up front; image i+1's transposes are
        # interleaved between image i's matmul groups so the PE never waits
        # on a bulk transpose phase.
        stg0 = emit_load(0)
        nxt = emit_load(1)
        for c in range(NCHUNK):
            emit_transpose(0, stg0, c)
        for i in range(BPC):
            oslab = out_pool.tile([GM, GROUPS * COUT], F32, tag="osb")
            done = 0
            for g in range(GROUPS):
                emit_group(i, g, oslab)
                if i + 1 < BPC:
                    # spread the 28 transposes of image i+1 over the groups
                    want = (g + 1) * NCHUNK // GROUPS
                    while done < want:
                        emit_transpose(i + 1, nxt, done)
                        done += 1
            # two big SWDGE stores: partitions 0-55 = even rows, 58-113 = odd
            dst_all = y_ap[i].rearrange("(g r) w c -> r w g c", r=2)
            for r in range(2):
                nc.gpsimd.dma_start(
                    out=dst_all[r],
                    in_=oslab[r * S : r * S + W, :].rearrange(
                        "p (g c) -> p g c", g=GROUPS
                    ),
                )
            if i + 1 < BPC:
                while done < NCHUNK:
                    emit_transpose(i + 1, nxt, done)
                    done += 1
                if i + 2 < BPC:
                    nxt = emit_load(i + 2)

    nc.compile()
    return nc


_CACHE = {}


def _get_nc(with_bias: bool):
    if with_bias not in _CACHE:
        _CACHE[with_bias] = _build(with_bias)
    return _CACHE[with_bias]


def kernel(prev_a, filter_w, filter_b):
    global LAST_RESULTS
    prev_a = np.ascontiguousarray(prev_a, dtype=np.float32)
    filter_w = np.ascontiguousarray(filter_w, dtype=np.float32)
    filter_b = np.ascontiguousarray(filter_b, dtype=np.float32).reshape(1, 1, 1, COUT)
    with_bias = bool(np.any(filter_b))
    nc = _get_nc(with_bias)
    in_maps = [
        {
            "prev_a": prev_a[c * BPC : (c + 1) * BPC],
            "filter_w": filter_w,
            "filter_b": filter_b,
        }
        for c in range(N_CORES)
    ]
    trace = os.environ.get("KERNEL_TRACE") == "1"
    res = run_bass_kernel_spmd(nc, in_maps, list(range(N_CORES)), trace=trace)
    LAST_RESULTS = res
    return np.concatenate([res.results[c]["out"] for c in range(N_CORES)], axis=0)


# revision 13
# speedup vs baseline: 2.3709x; 1.1139x over previous
"""3x3 same-conv (NHWC, 32x56x56x128 -> 32x56x56x256) + bias + ReLU on 8 TRN2 cores.

Strategy: data-parallel over batch (4 images/core). Per core, the conv is
9 shifted matmuls accumulated in PSUM with Cin=128 as the contraction dim:
  out[q, cout] = relu( sum_tap XpT[:, q+off_tap]^T @ W[tap] + b )
where XpT is the zero-padded image held transposed in SBUF ([cin, 58*58]
flat padded pixels), built once per image via PE transposes. Anchors q are
processed 128 at a time over the flat padded index space; the 2/58 pad
columns produce garbage rows that are simply never stored. Matmuls run as
float32r (full-rate on TRN2 for N>=256) on bitcast fp32 data.
"""

import os
from contextlib import ExitStack

import numpy as np

import concourse.bass as bass
import concourse.bacc as bacc
import concourse.mybir as mybir
import concourse.tile as tile
from concourse.bass_utils import run_bass_kernel_spmd
from concourse.masks import make_identity

N_CORES = 8
B, H, W, CIN, COUT = 32, 56, 56, 128, 256
BPC = B // N_CORES            # images per core
S = W + 2                     # padded width (58)
PIMG = S * S                  # padded pixels per image (3364)
ANCH0 = S + 1                 # first valid anchor (59)
GROUPS = H // 2               # 28 row-pair anchor groups per image
GM = 2 * S                    # anchors per group (116: 2 padded rows, 4 junk)
SLAB_W = PIMG + 128           # per-image slab width incl. zero slop
RPC = 2                       # image rows per transpose chunk
CHUNK_PIX = RPC * W           # 112
NCHUNK = H // RPC             # 28

TAP_OFFS = [(dh - 1) * S + (dw - 1) for dh in range(3) for dw in range(3)]
F32 = mybir.dt.float32
F32R = mybir.dt.float32r
F16 = mybir.dt.float16

LAST_RESULTS = None

if os.environ.get("KERNEL_LDWOPT") == "1":
    # Experiment: let walrus run its LDWEIGHTS optimization pass.
    import concourse.bass_utils as _bu

    _orig_run_command = _bu.run_command

    def _patched_run_command(argv, **kw):
        argv = [
            "--enable-ldw-opt=true" if a == "--enable-ldw-opt=false" else a
            for a in argv
        ]
        return _orig_run_command(argv, **kw)

    _bu.run_command = _patched_run_command


def _build(with_bias: bool):
    nc = bacc.Bacc("TRN2", target_bir_lowering=False, debug=False)
    x_h = nc.declare_dram_parameter("prev_a", [BPC, H, W, CIN], F32, isOutput=False)
    w_h = nc.declare_dram_parameter("filter_w", [3, 3, CIN, COUT], F32, isOutput=False)
    b_h = nc.declare_dram_parameter("filter_b", [1, 1, 1, COUT], F32, isOutput=False)
    y_h = nc.declare_dram_parameter("out", [BPC, H, W, COUT], F32, isOutput=True)
    x_ap, w_ap, b_ap, y_ap = x_h.ap(), w_h.ap(), b_h.ap(), y_h.ap()

    with tile.TileContext(nc) as tc, ExitStack() as ctx:
        const_pool = ctx.enter_context(tc.tile_pool(name="const", bufs=1))
        xslab_pool = ctx.enter_context(tc.tile_pool(name="xslab", bufs=1))
        stage_pool = ctx.enter_context(tc.tile_pool(name="stage", bufs=2))
        out_pool = ctx.enter_context(tc.tile_pool(name="outsb", bufs=2))
        psum_mm = ctx.enter_context(
            tc.tile_pool(name="psmm", bufs=4, space=bass.MemorySpace.PSUM)
        )
        psum_tp = ctx.enter_context(
            tc.tile_pool(name="pstp", bufs=4, space=bass.MemorySpace.PSUM)
        )

        # Weights: [3,3,128,256] -> SBUF [cin=128, tap*256], rounded to fp32r
        wstage = const_pool.tile([CIN, 9 * COUT], F32, tag="wstage")
        nc.sync.dma_start(
            out=wstage[:].rearrange("k (t n) -> k t n", t=9),
            in_=w_ap.rearrange("a b k n -> (a b) k n").transpose([1, 0, 2]),
        )
        wslab = const_pool.tile([CIN, 9 * COUT], F16, tag="wslab")
        nc.vector.tensor_copy(wslab[:], wstage[:])

        identity = const_pool.tile([CHUNK_PIX, CHUNK_PIX], F16, tag="ident")
        make_identity(nc, identity[:])

        if with_bias:
            bias_st = const_pool.tile([1, COUT], F32, tag="bias_st")
            nc.sync.dma_start(
                out=bias_st[:], in_=b_ap.rearrange("a b c n -> (a b c) n")
            )
            bias_sb = const_pool.tile([1, COUT], F16, tag="bias")
            nc.vector.tensor_copy(bias_sb[:], bias_st[:])
            ones_sb = const_pool.tile([1, 128], F16, tag="ones")
            nc.gpsimd.memset(ones_sb[:], 1.0)

        # Per-image transposed padded slabs [cin, 58*58 (+slop)]
        xslabs = []
        for i in range(BPC):
            sl = xslab_pool.tile([CIN, SLAB_W], F16, tag=f"xs{i}")
            xslabs.append(sl)
            nc.vector.memset(sl[:, 0:S], 0.0)  # top pad row
            nc.vector.memset(sl[:, (H + 1) * S : PIMG], 0.0)  # bottom pad row
            mid = sl[:, S : (H + 1) * S].rearrange("p (r c) -> p r c", c=S)
            nc.vector.memset(mid[:, :, 0:1], 0.0)  # left pad col
            nc.vector.memset(mid[:, :, S - 1 : S], 0.0)  # right pad col
            nc.vector.memset(sl[:, PIMG:SLAB_W], 0.0)  # slop

        # Slab build steps (load image, PE-transpose 2-row chunks, copy into
        # slab), emitted lazily so they interleave with prior image's matmuls
        def emit_load(i):
            stg = stage_pool.tile([CHUNK_PIX, NCHUNK * CIN], F16, tag="stage")
            src = (
                x_ap[i]
                .rearrange("h w c -> (h w) c")
                .rearrange("(n p) c -> n p c", p=CHUNK_PIX)
                .transpose([1, 0, 2])
            )
            hn = NCHUNK // 2
            dstv = stg[:].rearrange("p (n c) -> p n c", n=NCHUNK)
            nc.gpsimd.dma_start(out=dstv[:, 0:hn, :], in_=src[:, 0:hn, :])
            nc.gpsimd.dma_start(out=dstv[:, hn:, :], in_=src[:, hn:, :])
            return stg

        def emit_transpose(i, stg, cidx):
            pst = psum_tp.tile([CIN, CHUNK_PIX], F16, tag="pst")
            nc.tensor.transpose(
                pst[:], stg[:, cidx * CIN : (cidx + 1) * CIN], identity[:]
            )
            dst = (
                xslabs[i][:, (RPC * cidx + 1) * S : (RPC * cidx + 1 + RPC) * S]
                .rearrange("p (r c) -> p r c", c=S)[:, :, 1 : 1 + W]
            )
            nc.scalar.activation(
                dst,
                pst[:].rearrange("p (r c) -> p r c", c=W),
                mybir.ActivationFunctionType.Copy,
            )

        def emit_group(i, g, oslab):
            # anchors = 116 contiguous padded positions covering output rows
            # (2g, 2g+1); partitions 56,57,114,115 are pad junk (never stored)
            q0 = (2 * g + 1) * S + 1
            ps = psum_mm.tile([GM, COUT], F32, tag="psmm")
            for t in range(9):
                w0 = q0 + TAP_OFFS[t]
                nc.tensor.matmul(
                    ps[:],
                    xslabs[i][:, w0 : w0 + GM],
                    wslab[:, t * COUT : (t + 1) * COUT],
                    start=(t == 0),
                    stop=(t == 8 and not with_bias),
                )
            if with_bias:
                nc.tensor.matmul(
                    ps[:], ones_sb[:1, :GM], bias_sb[:1, :], start=False, stop=True
                )
            nc.vector.tensor_scalar_max(
                oslab[:, g * COUT : (g + 1) * COUT], ps[:], 0.0
            )

        # Image 0's slab is built up front; image i+1's transposes are
        # interleaved between image i's matmul groups so the PE never waits
        # on a bulk transpose phase.
        stg0 = emit_load(0)
        nxt = emit_load(1)
        for c in range(NCHUNK):
            emit_transpose(0, stg0, c)
        for i in range(BPC):
            oslab = out_pool.tile([GM, GROUPS * COUT], F32, tag="osb")
            done = 0
            for g in range(GROUPS):
                emit_group(i, g, oslab)
                if i + 1 < BPC:
                    # spread the 28 transposes of image i+1 over the groups
                    want = (g + 1) * NCHUNK // GROUPS
                    while done < want:
                        emit_transpose(i + 1, nxt, done)
                        done += 1
            # SWDGE stores: partitions 0-55 = even rows, 58-113 = odd rows;
            # issued in group-chunks so they overlap the remaining compute
            dst_all = y_ap[i].rearrange("(g r) w c -> r w g c", r=2)
            for r in range(2):
                srcv = oslab[r * S : r * S + W, :].rearrange(
                    "p (g c) -> p g c", g=GROUPS
                )
                for q0 in range(0, GROUPS, 7):
                    q1 = min(q0 + 7, GROUPS)
                    nc.gpsimd.dma_start(
                        out=dst_all[r][:, q0:q1, :], in_=srcv[:, q0:q1, :]
                    )
            if i + 1 < BPC:
                while done < NCHUNK:
                    emit_transpose(i + 1, nxt, done)
                    done += 1
                if i + 2 < BPC:
                    nxt = emit_load(i + 2)

    nc.compile()
    return nc


_CACHE = {}


def _get_nc(with_bias: bool):
    if with_bias not in _CACHE:
        _CACHE[with_bias] = _build(with_bias)
    return _CACHE[with_bias]


def kernel(prev_a, filter_w, filter_b):
    global LAST_RESULTS
    prev_a = np.ascontiguousarray(prev_a, dtype=np.float32)
    filter_w = np.ascontiguousarray(filter_w, dtype=np.float32)
    filter_b = np.ascontiguousarray(filter_b, dtype=np.float32).reshape(1, 1, 1, COUT)
    with_bias = bool(np.any(filter_b))
    nc = _get_nc(with_bias)
    in_maps = [
        {
            "prev_a": prev_a[c * BPC : (c + 1) * BPC],
            "filter_w": filter_w,
            "filter_b": filter_b,
        }
        for c in range(N_CORES)
    ]
    trace = os.environ.get("KERNEL_TRACE") == "1"
    res = run_bass_kernel_spmd(nc, in_maps, list(range(N_CORES)), trace=trace)
    LAST_RESULTS = res
    return np.concatenate([res.results[c]["out"] for c in range(N_CORES)], axis=0)


# revision 14
# speedup vs baseline: 2.4056x; 1.0146x over previous
"""3x3 same-conv (NHWC, 32x56x56x128 -> 32x56x56x256) + bias + ReLU on 8 TRN2 cores.

Strategy: data-parallel over batch (4 images/core). Per core, the conv is
9 shifted matmuls accumulated in PSUM with Cin=128 as the contraction dim:
  out[q, cout] = relu( sum_tap XpT[:, q+off_tap]^T @ W[tap] + b )
where XpT is the zero-padded image held transposed in SBUF ([cin, 58*58]
flat padded pixels, fp16), built once per image via PE transposes fed by a
casting SWDGE load. Each matmul group covers 116 contiguous padded
positions = two output rows plus 4 pad-junk anchors that are computed but
never stored; the junk keeps every matmul operand a contiguous SBUF window
(one free dim) while the two per-image output stores skip those partitions
with a strided DMA. fp16 operands stream the PE at 1 col/cycle with the
self-loading LDWEIGHTS fully hidden behind the previous matmul.
"""

import os
from contextlib import ExitStack

import numpy as np

import concourse.bass as bass
import concourse.bacc as bacc
import concourse.mybir as mybir
import concourse.tile as tile
from concourse.bass_utils import run_bass_kernel_spmd
from concourse.masks import make_identity

N_CORES = 8
B, H, W, CIN, COUT = 32, 56, 56, 128, 256
BPC = B // N_CORES            # images per core
S = W + 2                     # padded width (58)
PIMG = S * S                  # padded pixels per image (3364)
ANCH0 = S + 1                 # first valid anchor (59)
GROUPS = H // 2               # 28 row-pair anchor groups per image
GM = 2 * S                    # anchors per group (116: 2 padded rows, 4 junk)
SLAB_W = PIMG + 128           # per-image slab width incl. zero slop
RPC = 2                       # image rows per transpose chunk
CHUNK_PIX = RPC * W           # 112
NCHUNK = H // RPC             # 28

TAP_OFFS = [(dh - 1) * S + (dw - 1) for dh in range(3) for dw in range(3)]
F32 = mybir.dt.float32
F32R = mybir.dt.float32r
F16 = mybir.dt.float16

LAST_RESULTS = None


def _build(with_bias: bool):
    nc = bacc.Bacc("TRN2", target_bir_lowering=False, debug=False)
    x_h = nc.declare_dram_parameter("prev_a", [BPC, H, W, CIN], F32, isOutput=False)
    w_h = nc.declare_dram_parameter("filter_w", [3, 3, CIN, COUT], F32, isOutput=False)
    b_h = nc.declare_dram_parameter("filter_b", [1, 1, 1, COUT], F32, isOutput=False)
    y_h = nc.declare_dram_parameter("out", [BPC, H, W, COUT], F32, isOutput=True)
    x_ap, w_ap, b_ap, y_ap = x_h.ap(), w_h.ap(), b_h.ap(), y_h.ap()

    with tile.TileContext(nc) as tc, ExitStack() as ctx:
        const_pool = ctx.enter_context(tc.tile_pool(name="const", bufs=1))
        xslab_pool = ctx.enter_context(tc.tile_pool(name="xslab", bufs=1))
        stage_pool = ctx.enter_context(tc.tile_pool(name="stage", bufs=2))
        out_pool = ctx.enter_context(tc.tile_pool(name="outsb", bufs=2))
        psum_mm = ctx.enter_context(
            tc.tile_pool(name="psmm", bufs=4, space=bass.MemorySpace.PSUM)
        )
        psum_tp = ctx.enter_context(
            tc.tile_pool(name="pstp", bufs=4, space=bass.MemorySpace.PSUM)
        )

        # Weights: [3,3,128,256] -> SBUF [cin=128, tap*256], rounded to fp32r
        wstage = const_pool.tile([CIN, 9 * COUT], F32, tag="wstage")
        nc.sync.dma_start(
            out=wstage[:].rearrange("k (t n) -> k t n", t=9),
            in_=w_ap.rearrange("a b k n -> (a b) k n").transpose([1, 0, 2]),
        )
        wslab = const_pool.tile([CIN, 9 * COUT], F16, tag="wslab")
        nc.vector.tensor_copy(wslab[:], wstage[:])

        identity = const_pool.tile([CHUNK_PIX, CHUNK_PIX], F16, tag="ident")
        make_identity(nc, identity[:])

        if with_bias:
            bias_st = const_pool.tile([1, COUT], F32, tag="bias_st")
            nc.sync.dma_start(
                out=bias_st[:], in_=b_ap.rearrange("a b c n -> (a b c) n")
            )
            bias_sb = const_pool.tile([1, COUT], F16, tag="bias")
            nc.vector.tensor_copy(bias_sb[:], bias_st[:])
            ones_sb = const_pool.tile([1, 128], F16, tag="ones")
            nc.gpsimd.memset(ones_sb[:], 1.0)

        # Per-image transposed padded slabs [cin, 58*58 (+slop)]
        xslabs = []
        for i in range(BPC):
            sl = xslab_pool.tile([CIN, SLAB_W], F16, tag=f"xs{i}")
            xslabs.append(sl)
            nc.vector.memset(sl[:, 0:S], 0.0)  # top pad row
            nc.vector.memset(sl[:, (H + 1) * S : PIMG], 0.0)  # bottom pad row
            mid = sl[:, S : (H + 1) * S].rearrange("p (r c) -> p r c", c=S)
            nc.vector.memset(mid[:, :, 0:1], 0.0)  # left pad col
            nc.vector.memset(mid[:, :, S - 1 : S], 0.0)  # right pad col
            nc.vector.memset(sl[:, PIMG:SLAB_W], 0.0)  # slop

        # Slab build steps (load image, PE-transpose 2-row chunks, copy into
        # slab), emitted lazily so they interleave with prior image's matmuls
        def emit_load(i):
            stg = stage_pool.tile([CHUNK_PIX, NCHUNK * CIN], F16, tag="stage")
            src = (
                x_ap[i]
                .rearrange("h w c -> (h w) c")
                .rearrange("(n p) c -> n p c", p=CHUNK_PIX)
                .transpose([1, 0, 2])
            )
            dstv = stg[:].rearrange("p (n c) -> p n c", n=NCHUNK)
            for c0 in range(0, NCHUNK, 7):
                c1 = min(c0 + 7, NCHUNK)
                nc.gpsimd.dma_start(out=dstv[:, c0:c1, :], in_=src[:, c0:c1, :])
            return stg

        def emit_transpose(i, stg, cidx):
            pst = psum_tp.tile([CIN, CHUNK_PIX], F16, tag="pst")
            nc.tensor.transpose(
                pst[:], stg[:, cidx * CIN : (cidx + 1) * CIN], identity[:]
            )
            dst = (
                xslabs[i][:, (RPC * cidx + 1) * S : (RPC * cidx + 1 + RPC) * S]
                .rearrange("p (r c) -> p r c", c=S)[:, :, 1 : 1 + W]
            )
            nc.scalar.activation(
                dst,
                pst[:].rearrange("p (r c) -> p r c", c=W),
                mybir.ActivationFunctionType.Copy,
            )

        def emit_group(i, g, oslab):
            # anchors = 116 contiguous padded positions covering output rows
            # (2g, 2g+1); partitions 56,57,114,115 are pad junk (never stored)
            q0 = (2 * g + 1) * S + 1
            ps = psum_mm.tile([GM, COUT], F32, tag="psmm")
            for t in range(9):
                w0 = q0 + TAP_OFFS[t]
                nc.tensor.matmul(
                    ps[:],
                    xslabs[i][:, w0 : w0 + GM],
                    wslab[:, t * COUT : (t + 1) * COUT],
                    start=(t == 0),
                    stop=(t == 8 and not with_bias),
                )
            if with_bias:
                nc.tensor.matmul(
                    ps[:], ones_sb[:1, :GM], bias_sb[:1, :], start=False, stop=True
                )
            nc.vector.tensor_scalar_max(
                oslab[:, g * COUT : (g + 1) * COUT], ps[:], 0.0
            )

        # Image 0's slab is built up front; image i+1's transposes are
        # interleaved between image i's matmul groups so the PE never waits
        # on a bulk transpose phase.
        stg0 = emit_load(0)
        nxt = emit_load(1)
        for c in range(NCHUNK):
            emit_transpose(0, stg0, c)
        for i in range(BPC):
            oslab = out_pool.tile([GM, GROUPS * COUT], F32, tag="osb")
            done = 0
            for g in range(GROUPS):
                emit_group(i, g, oslab)
                if i + 1 < BPC:
                    # spread the 28 transposes of image i+1 over the groups
                    want = (g + 1) * NCHUNK // GROUPS
                    while done < want:
                        emit_transpose(i + 1, nxt, done)
                        done += 1
            # SWDGE stores: partitions 0-55 = even rows, 58-113 = odd rows;
            # issued in group-chunks so they overlap the remaining compute
            dst_all = y_ap[i].rearrange("(g r) w c -> r w g c", r=2)
            for r in range(2):
                srcv = oslab[r * S : r * S + W, :].rearrange(
                    "p (g c) -> p g c", g=GROUPS
                )
                for q0 in range(0, GROUPS, 4):
                    q1 = min(q0 + 4, GROUPS)
                    nc.gpsimd.dma_start(
                        out=dst_all[r][:, q0:q1, :], in_=srcv[:, q0:q1, :]
                    )
            if i + 1 < BPC:
                while done < NCHUNK:
                    emit_transpose(i + 1, nxt, done)
                    done += 1
                if i + 2 < BPC:
                    nxt = emit_load(i + 2)

    nc.compile()
    return nc


_CACHE = {}


def _get_nc(with_bias: bool):
    if with_bias not in _CACHE:
        _CACHE[with_bias] = _build(with_bias)
    return _CACHE[with_bias]


def kernel(prev_a, filter_w, filter_b):
    global LAST_RESULTS
    prev_a = np.ascontiguousarray(prev_a, dtype=np.float32)
    filter_w = np.ascontiguousarray(filter_w, dtype=np.float32)
    filter_b = np.ascontiguousarray(filter_b, dtype=np.float32).reshape(1, 1, 1, COUT)
    with_bias = bool(np.any(filter_b))
    nc = _get_nc(with_bias)
    in_maps = [
        {
            "prev_a": prev_a[c * BPC : (c + 1) * BPC],
            "filter_w": filter_w,
            "filter_b": filter_b,
        }
        for c in range(N_CORES)
    ]
    trace = os.environ.get("KERNEL_TRACE") == "1"
    res = run_bass_kernel_spmd(nc, in_maps, list(range(N_CORES)), trace=trace)
    LAST_RESULTS = res
    return np.concatenate([res.results[c]["out"] for c in range(N_CORES)], axis=0)
